# revision 1
# baseline (speedup 1.0000x reference)
"""Trainium2 Bass kernel for nn_BAC_15152644620305.

Per batch element (1 per NeuronCore, 8 cores):
  p_dense = relu(p @ W1 + b1); q_dense = relu(q @ W2 + b2)
  A = (p_dense @ q_dense.T) / sqrt(600)
  passage_aligned = softmax_rows(A) @ passage ; query_aligned = softmax_cols(A).T @ query
  6 factorization-machine heads on {concat, diff, mul} pairs -> [L, 3] x 2 outputs.

Implementation notes:
  - All heavy matmuls in bf16 (1 cyc/row on PE), fp32 PSUM accumulation.
  - Affinity computed in BOTH layouts (cheaper than transposing exp(A) on-chip);
    exp without max-subtraction (affinity values are in [0.1, 1.1]).
  - exp(A) stored as fp8e4m3 (softmax weights only -> negligible error, halves SBUF).
  - Softmax denominators ride along as an extra ones-column in the aligned matmuls'
    stationary operand, landing at an aligned output partition (96).
  - FM heads algebraically reduced: the x^2 @ V^2.T term needs only sum_k V_k^2;
    diff projections are linear combos of the qa/p projections; all per-head
    combination is done by one small stationary matmul per output chunk.
"""
import numpy as np

L_FULL = 2048
D = 600
U = 300
KFM = 5
N_CORES = 8
SCALE = float(1.0 / np.sqrt(np.float32(D)))

DCH = [(0, 128), (128, 128), (256, 128), (384, 128), (512, 88)]   # D chunks
UCH = [(0, 128), (128, 128), (256, 44)]                           # U chunks
ONES_COL = 608        # column in the 640-wide natural tile holding the ones
ONES_ROW = 96         # output partition where the denominator row lands
NATW = 640


def _emit(nc, L):
    import concourse.bass as bass
    import concourse.mybir as mybir
    import concourse.tile as tile
    from concourse.masks import make_identity
    from contextlib import ExitStack

    f32 = mybir.dt.float32
    bf16 = mybir.dt.bfloat16
    fp8 = mybir.dt.float8e4
    AF = mybir.ActivationFunctionType
    ds = bass.ds

    LT = L // 128               # l tiles
    NCW = min(512, L)           # moving-dim chunk width
    NCX = L // NCW              # chunks per L
    TG = 4 if LT % 4 == 0 else 1  # l-tiles per transpose psum batch

    x_d = nc.dram_tensor("x", [2, L, D], f32, kind="ExternalInput")
    wp_d = nc.dram_tensor("wpack", [10, 128, U], f32, kind="ExternalInput")
    sp_d = nc.dram_tensor("statp", [10, 128, 36], f32, kind="ExternalInput")
    c2_d = nc.dram_tensor("comb2", [128, 6], f32, kind="ExternalInput")
    bp_d = nc.dram_tensor("biasp", [128, 6], f32, kind="ExternalInput")
    w0_d = nc.dram_tensor("w0col", [3, 2], f32, kind="ExternalInput")
    out_d = nc.dram_tensor("out", [2, 3, L], f32, kind="ExternalOutput")

    with tile.TileContext(nc) as tc, ExitStack() as ctx:
        const = ctx.enter_context(tc.tile_pool(name="const", bufs=1))
        big = ctx.enter_context(tc.tile_pool(name="big", bufs=1))
        epool = ctx.enter_context(tc.tile_pool(name="epool", bufs=LT // 2))
        natp = ctx.enter_context(tc.tile_pool(name="natp", bufs=LT))
        nf32p = ctx.enter_context(tc.tile_pool(name="nf32p", bufs=6))
        stg = ctx.enter_context(tc.tile_pool(name="stg", bufs=2))
        fmt = ctx.enter_context(tc.tile_pool(name="fmt", bufs=4))
        sp = ctx.enter_context(tc.tile_pool(name="sp", bufs=2))
        rp = ctx.enter_context(tc.tile_pool(name="rp", bufs=2))
        ob = ctx.enter_context(tc.tile_pool(name="ob", bufs=1))
        ps = ctx.enter_context(tc.tile_pool(name="ps", bufs=8, space="PSUM"))

        def pst(p_cnt=128, w=NCW):
            return ps.tile([p_cnt, w], f32, tag="ps", name="pst")

        # ------- constants (packed loads on the scalar HWDGE queue) -------
        identb = const.tile([128, 128], bf16, tag="identb")
        make_identity(nc, identb)
        onesb = const.tile([128, 128], bf16, tag="onesb")
        nc.vector.memset(onesb[:], 1.0)
        w0sb = const.tile([3, 2], f32, tag="w0sb")
        nc.scalar.dma_start(w0sb[:], w0_d[:])

        wstg = stg.tile([128, 10 * U], f32, tag="stg_w", name="wstg", bufs=1)
        nc.scalar.dma_start(
            wstg[:].rearrange("p (t c) -> p t c", t=10),
            wp_d[:].rearrange("t p c -> p t c"))
        Wall = const.tile([128, 10 * U], bf16, tag="Wall")
        nc.vector.tensor_copy(Wall[:], wstg[:])
        Wsb = [[Wall[:, ds((t * 5 + k) * U, U)] for k in range(5)]
               for t in range(2)]

        sstg = stg.tile([128, 360], f32, tag="stg_s", name="sstg", bufs=1)
        nc.scalar.dma_start(
            sstg[:].rearrange("p (t c) -> p t c", t=10),
            sp_d[:].rearrange("t p c -> p t c"))
        Sall = const.tile([128, 360], bf16, tag="Sall")
        nc.vector.tensor_copy(Sall[:], sstg[:])
        stat = [[Sall[:, ds((s * 5 + k) * 36, 36)] for k in range(5)]
                for s in range(2)]

        cstg = stg.tile([128, 6], f32, tag="stg_c", name="cstg", bufs=1)
        nc.scalar.dma_start(cstg[:], c2_d[:])
        cb2 = const.tile([128, 6], bf16, tag="cb2")
        nc.vector.tensor_copy(cb2[:], cstg[:])

        bsb = const.tile([128, 6], f32, tag="bsb")
        nc.scalar.dma_start(bsb[:], bp_d[:])

        # ---------------- phase 1: transpose inputs -> pT/qT (bf16 [d, L]) ----
        xT = [[], []]
        for t in range(2):
            for k in range(len(DCH)):
                xT[t].append(big.tile([128, L], bf16, tag=f"xT{t}_{k}",
                                      name=f"xT{t}_{k}"))
        # phase 1+2 interleaved per l-group: transpose inputs -> pT/qT, then
        # the dense matmuls for that group's columns (keeps PE fed during the
        # next group's DMA + cast)
        # u-chunks 0,1 live as one fp8 PAIR tile (DoubleRow operand for the
        # affinity matmuls); the 44-row chunk 2 stays bf16 (base-0 + base-64)
        dTP = [big.tile([128, 2, L], fp8, tag=f"dTP{t}", name=f"dTP{t}")
               for t in range(2)]
        dT2 = [big.tile([128, L], bf16, tag=f"dT2{t}", name=f"dT2{t}")
               for t in range(2)]
        nats = [[None] * (LT // 2) for _ in range(2)]
        for g in range(LT // TG):
            gw = TG * 128
            for t in range(2):
                # 2 d-chunks per bf16 psum tile (same 2KB bank footprint as
                # one f32 slot) -> 3 slots instead of 5, more slot headroom
                # for the dense accumulators and the next group's transposes
                pjs2 = [ps.tile([128, 2 * NCW], bf16, tag="ps", name="pjs")
                        for _ in range((len(DCH) + 1) // 2)]
                pjs = [pjs2[k // 2][:, ds((k % 2) * NCW, NCW)]
                       for k in range(len(DCH))]
                for ii in range(TG):
                    i = g * TG + ii
                    nf = nf32p.tile([128, D], f32, tag="nf", name="nf")
                    eng = nc.sync if (g == 0 or i % 2 == 0) else nc.scalar
                    eng.dma_start(nf[:], x_d[t, ds(i * 128, 128), :])
                    nfb = nf32p.tile([128, D], bf16, tag="nfb", name="nfb")
                    nc.vector.tensor_copy(nfb[:], nf[:])
                    # build the fp8 natural-layout pair tile (DoubleRow operand
                    # of the aligned matmuls) from the same load
                    pi, j = i // 2, i % 2
                    if j == 0:
                        nats[t][pi] = natp.tile([128, 2, NATW], fp8, tag="nat",
                                                name=f"nat{t}_{pi}")
                        nc.gpsimd.memset(nats[t][pi][:], 0.0)
                    nt = nats[t][pi]
                    # split between DVE and ACT to balance this phase
                    if j == 0:
                        nc.vector.tensor_copy(nt[:, j, 0:D], nf[:])
                    else:
                        nc.scalar.copy(nt[:, j, 0:D], nf[:])
                    nc.gpsimd.memset(nt[:, j, ONES_COL:ONES_COL + 1], 1.0)
                    for k, (doff, dcnt) in enumerate(DCH):
                        nc.tensor.transpose(
                            pjs[k][:dcnt, ds(ii * 128, 128)],
                            nfb[:, ds(doff, dcnt)], identb[:])
                for k, (doff, dcnt) in enumerate(DCH):
                    # alternate engines: balances DVE (casts) vs ACT this phase
                    if k % 2 == 0:
                        nc.vector.tensor_copy(xT[t][k][:dcnt, ds(g * gw, gw)],
                                              pjs[k][:dcnt, ds(0, gw)])
                    else:
                        nc.scalar.copy(xT[t][k][:dcnt, ds(g * gw, gw)],
                                       pjs[k][:dcnt, ds(0, gw)])
            if gw == NCW:
                for t in range(2):
                    for m, (uoff, ucnt) in enumerate(UCH[:2]):
                        acc = pst()
                        for k, (doff, dcnt) in enumerate(DCH):
                            nc.tensor.matmul(
                                acc[:ucnt, :],
                                Wsb[t][k][:dcnt, ds(uoff, ucnt)],
                                xT[t][k][:dcnt, ds(g * NCW, NCW)],
                                start=(k == 0), stop=(k == len(DCH) - 1))
                        nc.scalar.activation(
                            dTP[t][:, m, ds(g * NCW, NCW)], acc[:ucnt, :],
                            AF.Relu, bias=bsb[:ucnt, t * 3 + m: t * 3 + m + 1])
                # the 44-row M-chunk: both tensors' matmuls in concurrent
                # col-groups (0 and 64) of one psum tile
                uoff, ucnt = UCH[2]
                acc2 = pst()
                for k, (doff, dcnt) in enumerate(DCH):
                    fl = (k == 0, k == len(DCH) - 1)
                    nc.tensor.matmul(
                        acc2[0:ucnt, :],
                        Wsb[0][k][:dcnt, ds(uoff, ucnt)],
                        xT[0][k][:dcnt, ds(g * NCW, NCW)],
                        start=fl[0], stop=fl[1], tile_position=(0, 0),
                        skip_group_check=True)
                    nc.tensor.matmul(
                        acc2[64:64 + ucnt, :],
                        Wsb[1][k][:dcnt, ds(uoff, ucnt)],
                        xT[1][k][:dcnt, ds(g * NCW, NCW)],
                        start=fl[0], stop=fl[1], tile_position=(0, 64),
                        skip_group_check=True)
                for t in range(2):
                    pb = t * 64
                    for dst in (0, 64):
                        # evict to base 0 (affinity k2 slice) and base 64
                        # (its row-pair partner slice)
                        nc.scalar.activation(
                            dT2[t][dst:dst + ucnt, ds(g * NCW, NCW)],
                            acc2[pb:pb + ucnt, :], AF.Relu,
                            bias=bsb[:ucnt, t * 3 + 2: t * 3 + 3])
        if TG * 128 != NCW:
            for t in range(2):
                for m, (uoff, ucnt) in enumerate(UCH):
                    for nx in range(NCX):
                        acc = pst()
                        for k, (doff, dcnt) in enumerate(DCH):
                            nc.tensor.matmul(
                                acc[:ucnt, :],
                                Wsb[t][k][:dcnt, ds(uoff, ucnt)],
                                xT[t][k][:dcnt, ds(nx * NCW, NCW)],
                                start=(k == 0), stop=(k == len(DCH) - 1))
                        if m < 2:
                            nc.scalar.activation(
                                dTP[t][:, m, ds(nx * NCW, NCW)], acc[:ucnt, :],
                                AF.Relu,
                                bias=bsb[:ucnt, t * 3 + m: t * 3 + m + 1])
                        else:
                            for dst in (0, 64):
                                nc.scalar.activation(
                                    dT2[t][dst:dst + ucnt, ds(nx * NCW, NCW)],
                                    acc[:ucnt, :], AF.Relu,
                                    bias=bsb[:ucnt, t * 3 + m: t * 3 + m + 1])

        # helpers ------------------------------------------------------------
        def affinity_to_E(dPa, d2a, dPb, d2b, tagged, interleave=()):
            """E[i] tiles [128, L] fp8 = exp(SCALE * lhs.T @ rhs) per l-tile."""
            E = []
            hooks = dict(interleave)
            for i in range(LT):
                if i in hooks:
                    hooks[i]()
                if i % 2 == 0:
                    e = epool.tile([128, 2, L], fp8, tag="E",
                                   name=f"E{tagged}_{i}")
                    E.append(e)
                else:
                    e = E[-1]
                ej = i % 2
                isl = ds(i * 128, 128)
                DRm = mybir.MatmulPerfMode.DoubleRow
                if NCX % 2 == 0:
                    for nx0 in range(0, NCX, 2):
                        accs = (pst(), pst())
                        for j in (0, 1):
                            nsl = ds((nx0 + j) * NCW, NCW)
                            # u-chunks 0+1 in one fp8 DoubleRow pass
                            nc.tensor.matmul(
                                accs[j][:, :], dPa[:, :, isl],
                                dPb[:, :, nsl],
                                start=True, stop=False, perf_mode=DRm)
                        # 44-row K chunk (bf16): the two N-chunks' matmuls go
                        # to disjoint PE row-groups and run concurrently
                        nc.tensor.matmul(
                            accs[0][:, :], d2a[0:44, isl],
                            d2b[0:44, ds(nx0 * NCW, NCW)],
                            start=False, stop=True, tile_position=(0, 0))
                        nc.tensor.matmul(
                            accs[1][:, :], d2a[64:108, isl],
                            d2b[64:108, ds((nx0 + 1) * NCW, NCW)],
                            start=False, stop=True, tile_position=(64, 0))
                        for j in (0, 1):
                            nsl = ds((nx0 + j) * NCW, NCW)
                            nc.scalar.activation(e[:, ej, nsl], accs[j][:, :],
                                                 AF.Exp, scale=SCALE)
                else:
                    for nx in range(NCX):
                        acc = pst()
                        nsl = ds(nx * NCW, NCW)
                        nc.tensor.matmul(acc[:, :], dPa[:, :, isl],
                                         dPb[:, :, nsl],
                                         start=True, stop=False,
                                         perf_mode=DRm)
                        nc.tensor.matmul(acc[:, :], d2a[0:44, isl],
                                         d2b[0:44, nsl],
                                         start=False, stop=True)
                        nc.scalar.activation(e[:, ej, nsl],
                                             acc[:, :], AF.Exp, scale=SCALE)
            return E

        def aligned_T(nats, E, side_tag):
            """alT tiles [d,L] bf16 = normalized aligned.T, via ones-row trick."""
            alT = [big.tile([128, L], bf16, tag=f"alT{k}", name=f"alT{side_tag}{k}")
                   for k in range(len(DCH))]
            R = big.tile([128, L], bf16, tag="R", name=f"R{side_tag}")
            NP = LT // 2
            DR = mybir.MatmulPerfMode.DoubleRow
            # pass A: last d-chunk (88 rows) + ones row at partition 96
            ps4 = [pst() for _ in range(NCX)]
            for pi in range(NP):
                for nx in range(NCX):
                    nc.tensor.matmul(ps4[nx][:, :],
                                     nats[pi][:, :, ds(512, 128)],
                                     E[pi][:, :, ds(nx * NCW, NCW)],
                                     start=(pi == 0), stop=(pi == NP - 1),
                                     perf_mode=DR)
            for nx in range(NCX):
                rr = rp.tile([128, NCW], f32, tag="rr", name="rr")
                nc.vector.reciprocal(rr[ONES_ROW:ONES_ROW + 1, :],
                                     ps4[nx][ONES_ROW:ONES_ROW + 1, :])
                rrb = rp.tile([128, NCW], bf16, tag="rrb", name="rrb")
                nc.scalar.copy(rrb[ONES_ROW:ONES_ROW + 1, :],
                               rr[ONES_ROW:ONES_ROW + 1, :])
                bc = pst()
                nc.tensor.matmul(bc[:, :], onesb[ONES_ROW:ONES_ROW + 1, 0:128],
                                 rrb[ONES_ROW:ONES_ROW + 1, :],
                                 start=True, stop=True,
                                 tile_position=(ONES_ROW, 0))
                nc.scalar.copy(R[:, ds(nx * NCW, NCW)], bc[:, :])
                nc.vector.tensor_mul(alT[4][0:88, ds(nx * NCW, NCW)],
                                     ps4[nx][0:88, :], R[0:88, ds(nx * NCW, NCW)])
            # passes B, C: d-chunks 0..3, two at a time
            for mm0 in (0, 2):
                accs = {}
                for m in (mm0, mm0 + 1):
                    for nx in range(NCX):
                        accs[(m, nx)] = pst()
                for pi in range(NP):
                    for m in (mm0, mm0 + 1):
                        for nx in range(NCX):
                            nc.tensor.matmul(accs[(m, nx)][:, :],
                                             nats[pi][:, :, ds(m * 128, 128)],
                                             E[pi][:, :, ds(nx * NCW, NCW)],
                                             start=(pi == 0),
                                             stop=(pi == NP - 1),
                                             perf_mode=DR)
                for m in (mm0, mm0 + 1):
                    for nx in range(NCX):
                        nc.vector.tensor_mul(alT[m][:, ds(nx * NCW, NCW)],
                                             accs[(m, nx)][:, :],
                                             R[:, ds(nx * NCW, NCW)])
            return alT

        def fm_proj(s, xTs, bTs):
            """FM projection matmuls for one side; returns live PSUM groups.

            d-chunk-outer loop: the elementwise temps are built full-width once
            per chunk (fewer DVE ops, deeper PE overlap); all four N-chunks'
            projection groups accumulate simultaneously (8 PSUM banks).
            """
            P1s = [ps.tile([128, NCW], f32, tag="ps", name="P1")
                   for _ in range(NCX)]
            P2s = [ps.tile([128, NCW], f32, tag="ps", name="P2")
                   for _ in range(NCX)]
            nk = len(DCH)
            for k, (doff, dcnt) in enumerate(DCH):
                x_fl = xTs[k][:dcnt, :]
                b_fl = bTs[k][:dcnt, :]
                tx2 = fmt.tile([128, L], bf16, tag="fmt", name="tx2")
                tb2 = fmt.tile([128, L], bf16, tag="fmt", name="tb2")
                txm = fmt.tile([128, L], bf16, tag="fmt", name="txm")
                txm2 = fmt.tile([128, L], bf16, tag="fmt", name="txm2")
                nc.vector.tensor_mul(tx2[:dcnt, :], x_fl, x_fl)
                nc.vector.tensor_mul(tb2[:dcnt, :], b_fl, b_fl)
                nc.vector.tensor_mul(txm[:dcnt, :], x_fl, b_fl)
                nc.vector.tensor_mul(txm2[:dcnt, :], txm[:dcnt, :],
                                     txm[:dcnt, :])
                st = stat[s][k]
                fl = (k == 0, k == nk - 1)
                for nx in range(NCX):
                    nsl = ds(nx * NCW, NCW)
                    P1, P2 = P1s[nx], P2s[nx]
                    nc.tensor.matmul(P1[0:12, :], st[:dcnt, 0:12],
                                     xTs[k][:dcnt, nsl],
                                     start=fl[0], stop=fl[1],
                                     tile_position=(0, 0),
                                     skip_group_check=True)
                    nc.tensor.matmul(P1[32:44, :], st[:dcnt, 12:24],
                                     bTs[k][:dcnt, nsl],
                                     start=fl[0], stop=fl[1],
                                     tile_position=(0, 32),
                                     skip_group_check=True)
                    nc.tensor.matmul(P1[64:65, :], st[:dcnt, 35:36],
                                     txm2[:dcnt, nsl], start=fl[0], stop=fl[1],
                                     tile_position=(0, 64),
                                     skip_group_check=True)
                    nc.tensor.matmul(P2[0:2, :], st[:dcnt, 24:26],
                                     tx2[:dcnt, nsl], start=fl[0], stop=fl[1],
                                     tile_position=(0, 0),
                                     skip_group_check=True)
                    nc.tensor.matmul(P2[32:34, :], st[:dcnt, 26:28],
                                     tb2[:dcnt, nsl], start=fl[0], stop=fl[1],
                                     tile_position=(0, 32),
                                     skip_group_check=True)
                    nc.tensor.matmul(P2[64:71, :], st[:dcnt, 28:35],
                                     txm[:dcnt, nsl], start=fl[0], stop=fl[1],
                                     tile_position=(0, 64),
                                     skip_group_check=True)
            return P1s, P2s

        def fm_comb(s, P1s, P2s, nx):
            """Evict + combine one N-chunk of one side's FM groups."""
            if True:
                nsl = ds(nx * NCW, NCW)
                P1, P2 = P1s[nx], P2s[nx]
                # pack group evictions at 32-aligned partition offsets so the
                # whole combine is 2 matmuls: S1 = [X@0, B@32, X2@64, B2@96],
                # S2 = [M@0, M2@32, TQ@64, TQM@96]
                S1 = sp.tile([128, NCW], bf16, tag="S1", name="S1")
                S2 = sp.tile([128, NCW], bf16, tag="S2", name="S2")
                nc.vector.memset(S1[:], 0.0)
                nc.vector.memset(S2[:], 0.0)
                # split evictions ACT/DVE so the S-build runs in parallel
                nc.scalar.copy(S1[0:12, :], P1[0:12, :])
                nc.scalar.copy(S1[32:44, :], P1[32:44, :])
                nc.vector.tensor_copy(S1[64:66, :], P2[0:2, :])
                nc.vector.tensor_copy(S1[96:98, :], P2[32:34, :])
                nc.vector.tensor_copy(S2[0:7, :], P2[64:71, :])
                nc.vector.tensor_copy(S2[32:33, :], P1[64:65, :])
                # B-group Vd columns carry -Vd, so diff quads are also an add.
                # in0 from PSUM: two SBUF inputs must share a base partition.
                TA = sp.tile([10, NCW], f32, tag="TA", name="TA")
                nc.vector.tensor_add(TA[0:10, :], P1[0:10, :], S1[32:42, :])
                nc.scalar.activation(S2[64:74, :], TA[:, :], AF.Square)
                nc.scalar.activation(S2[96:101, :], S2[0:5, :], AF.Square)
                cps = ps.tile([3, NCW], f32, tag="ps", name="cps")
                nc.tensor.matmul(cps[:, :], cb2[0:98, 0:3], S1[0:98, :],
                                 start=True, stop=False)
                nc.tensor.matmul(cps[:, :], cb2[0:101, 3:6], S2[0:101, :],
                                 start=False, stop=True)
                o = ob.tile([3, NCW], f32, tag="ob", name="o")
                nc.scalar.activation(o[:, :], cps[:, :], AF.Identity,
                                     bias=w0sb[:, s:s + 1])
                nc.sync.dma_start(out_d[s, :, nsl], o[:, :])

        # ---------------- main flow ----------------
        E1 = affinity_to_E(dTP[0], dT2[0], dTP[1], dT2[1], "1")     # E1[p-tile][p, q]
        qaT = aligned_T(nats[1], E1, "q")          # query_aligned.T
        P1s, P2s = fm_proj(0, qaT, xT[0])         # passage-side projections
        # interleave passage-side combines with A2: the combines free PSUM
        # banks that A2 then takes, and A2's matmuls keep PE fed while the
        # combines' ACT/DVE S-builds run
        combs = [lambda nx=nx: fm_comb(0, P1s, P2s, nx) for nx in range(NCX)]
        for c in combs[:2]:
            c()
        E2 = affinity_to_E(dTP[1], dT2[1], dTP[0], dT2[0], "2",
                           interleave=list(enumerate(combs[2:], start=1)))
        paT = aligned_T(nats[0], E2, "p")          # passage_aligned.T
        P1s1, P2s1 = fm_proj(1, paT, xT[1])       # query-side projections
        for nx in range(NCX):
            fm_comb(1, P1s1, P2s1, nx)


def _host_prep(W1, b1, W2, b2, cat_w0, cat_w, cat_V, dm_w0, dm_w, dm_V):
    stat = np.zeros((2, D, 36), np.float32)
    for s in range(2):
        ci, di, mi = s, s, s + 2
        Va = cat_V[ci][:, :D]
        Vb = cat_V[ci][:, D:]
        Vd = dm_V[di]
        Vm = dm_V[mi]
        stat[s, :, 0:5] = Va.T
        stat[s, :, 5:10] = Vd.T
        stat[s, :, 10] = cat_w[ci, :D]
        stat[s, :, 11] = dm_w[di]
        stat[s, :, 12:17] = Vb.T
        stat[s, :, 17:22] = -Vd.T   # negated: quad build is then a single add
        stat[s, :, 22] = cat_w[ci, D:]
        stat[s, :, 23] = dm_w[di]
        stat[s, :, 24] = (Va ** 2).sum(0)
        stat[s, :, 25] = (Vd ** 2).sum(0)
        stat[s, :, 26] = (Vb ** 2).sum(0)
        stat[s, :, 27] = (Vd ** 2).sum(0)
        stat[s, :, 28:33] = Vm.T
        stat[s, :, 33] = dm_w[mi]
        stat[s, :, 34] = (Vd ** 2).sum(0)
        stat[s, :, 35] = (Vm ** 2).sum(0)

    # packed combine matrices: S1 = [X@0, B@32, X2@64, B2@96],
    # S2 = [M@0, M2@32, TQ@64, TQM@96]
    comb2 = np.zeros((128, 6), np.float32)
    C1, C2 = comb2[:, 0:3], comb2[:, 3:6]
    C1[10, 0] = 1.0     # x@w_cat -> c_cat
    C1[11, 1] = 1.0     # x@w_d -> c_diff
    C1[32 + 10, 0] = 1.0
    C1[32 + 11, 1] = -1.0
    C1[64, 0] = -0.5    # x2@u_cat
    C1[65, 1] = -0.5    # x2@u_d
    C1[96, 0] = -0.5    # b2@u_cat
    C1[97, 1] = -0.5    # b2@u_d
    C2[5, 2] = 1.0      # mul@w_m
    C2[6, 1] = 1.0      # mul@u_d (from -0.5 * -2)
    C2[32, 2] = -0.5    # mul2@u_m
    C2[64:69, 0] = 0.5  # cat quads
    C2[69:74, 1] = 0.5  # diff quads
    C2[96:101, 2] = 0.5  # mul quads

    # packed per-d-chunk weights / stationaries / bias
    wpack = np.zeros((10, 128, U), np.float32)
    statp = np.zeros((10, 128, 36), np.float32)
    for t, W in enumerate((W1, W2)):
        for k, (doff, dcnt) in enumerate(DCH):
            wpack[t * 5 + k, :dcnt] = W[doff:doff + dcnt]
    for s in range(2):
        for k, (doff, dcnt) in enumerate(DCH):
            statp[s * 5 + k, :dcnt] = stat[s, doff:doff + dcnt]

    biasp = np.zeros((128, 6), np.float32)
    for t, b in enumerate((b1, b2)):
        for m, (uoff, ucnt) in enumerate(UCH):
            biasp[:ucnt, t * 3 + m] = b[uoff:uoff + ucnt]

    w0col = np.zeros((3, 2), np.float32)
    for s in range(2):
        w0col[0, s] = cat_w0[s, 0]
        w0col[1, s] = dm_w0[s, 0]
        w0col[2, s] = dm_w0[s + 2, 0]
    return wpack, statp, comb2, biasp, w0col


_PROG = None


def _get_prog():
    global _PROG
    if _PROG is None:
        from concourse import bacc
        nc = bacc.Bacc(None, target_bir_lowering=False)
        _emit(nc, L_FULL)
        nc.finalize()
        _PROG = nc
    return _PROG


def _in_maps(stack_input, W1, b1, W2, b2, fm_cat_w0, fm_cat_w, fm_cat_V,
             fm_dm_w0, fm_dm_w, fm_dm_V):
    f = lambda a: np.ascontiguousarray(np.asarray(a, np.float32))
    stack_input = f(stack_input)
    wpack, statp, comb2, biasp, w0col = _host_prep(
        f(W1), f(b1), f(W2), f(b2), f(fm_cat_w0), f(fm_cat_w), f(fm_cat_V),
        f(fm_dm_w0), f(fm_dm_w), f(fm_dm_V))
    common = {"wpack": wpack, "statp": statp, "comb2": comb2, "biasp": biasp,
              "w0col": w0col}
    return [dict(common, x=np.ascontiguousarray(stack_input[:, b]))
            for b in range(N_CORES)]


def kernel(stack_input, W1, b1, W2, b2, fm_cat_w0, fm_cat_w, fm_cat_V,
           fm_dm_w0, fm_dm_w, fm_dm_V):
    from concourse.bass_utils import run_bass_kernel_spmd

    in_maps = _in_maps(stack_input, W1, b1, W2, b2, fm_cat_w0, fm_cat_w,
                       fm_cat_V, fm_dm_w0, fm_dm_w, fm_dm_V)
    nc = _get_prog()
    res = run_bass_kernel_spmd(nc, in_maps, core_ids=list(range(N_CORES)))
    outs = [r["out"] for r in res.results]            # each [2, 3, L]
    fp = np.stack([o[0].T for o in outs]).astype(np.float32)   # [8, L, 3]
    fq = np.stack([o[1].T for o in outs]).astype(np.float32)
    return fp, fq



# revision 77
# speedup vs baseline: 1.2354x; 1.2354x over previous
"""Trainium2 Bass kernel for nn_BAC_15152644620305 (v2).

Per batch element (1 per NeuronCore, 8 cores):
  p_dense = relu(p @ W1 + b1); q_dense = relu(q @ W2 + b2)
  A = (p_dense @ q_dense.T) / sqrt(600)
  passage_aligned = softmax_rows(A) @ passage ; query_aligned = softmax_cols(A).T @ query
  6 factorization-machine heads on {concat, diff, mul} pairs -> [L, 3] x 2 outputs.

v2 implementation notes (cost-model-driven):
  - Everything that tolerates fp8 runs as fp8 DoubleRow matmuls (0.5 cyc/row):
    dense (W and xT d-paired), both affinity layouts (u-paired, incl. the
    44-row tail), aligned (nats/E as before), and the FM square/mul planes
    (x^2,b^2) and (xb,xb^2) paired as DR row-pairs with disjoint stationary
    column groups.
  - Precision-critical FM paths stay bf16: the x/b linear+quad projections
    (fp8 there costs 3e-2 rel err) and the u = sum_k V_k^2 stationaries are
    pre-scaled x256 into fp8 normal range (denormals cost 2.5e-2 otherwise),
    with the 1/256 folded into the f32->bf16 combine matrix.
  - xb^2 = x^2 * b^2 (product of already-built fp8 planes), no extra square.
  - FM combine: 4 matmul groups per PSUM tile at 32-aligned offsets with
    zero-padded stationaries so the whole [128, L] block evicts in 4 copies,
    then TA-add + 2 squares + one [128->3] combine matmul per side.
  - exp at N=1024 (two ACT ops per l-tile); PSUM: 2x[128,1024] "acc" ring +
    4x[128,512] "fmp" ring.
  - nats junk columns get narrow memsets only; elementwise planes spread
    across DVE/ACT/Pool (scalar_tensor_tensor on gpsimd).
"""
import numpy as np

L_FULL = 2048
D = 600
U = 300
KFM = 5
N_CORES = 8
SCALE = float(1.0 / np.sqrt(np.float32(D)))
USC = 256.0           # fp8 pre-scale for the u = sum V^2 stationaries

DCH = [(0, 128), (128, 128), (256, 128), (384, 128), (512, 88)]   # D chunks
ONES_COL = 96         # ones column within the 128-wide natural tail tile
ONES_ROW = 96         # denominator row in the pass-A psum
NATW = 640


def _emit(nc, L):
    import concourse.bass as bass
    import concourse.mybir as mybir
    import concourse.tile as tile
    from concourse.masks import make_identity
    from contextlib import ExitStack

    f32 = mybir.dt.float32
    bf16 = mybir.dt.bfloat16
    fp8 = mybir.dt.float8e4
    AF = mybir.ActivationFunctionType
    ALU = mybir.AluOpType
    ds = bass.ds
    DR = mybir.MatmulPerfMode.DoubleRow

    LT = L // 128               # 16 l-tiles
    NP = LT // 2                # 8 pair tiles
    TG = 4                      # l-tiles per transpose group
    NG = LT // TG               # 4 groups (512 cols each)
    GW = TG * 128               # 512
    NH = L // 1024              # halves (2)

    x_d = nc.dram_tensor("x", [2, L, D], f32, kind="ExternalInput")
    wp_d = nc.dram_tensor("wpair", [3, 128, 2, 1024], f32, kind="ExternalInput")
    pa_d = nc.dram_tensor("pastat", [10, 128, 64], f32, kind="ExternalInput")
    pbc_d = nc.dram_tensor("pbcstat", [10, 128, 2, 64], f32, kind="ExternalInput")
    c2_d = nc.dram_tensor("comb2", [128, 6], f32, kind="ExternalInput")
    bp_d = nc.dram_tensor("biasp", [128, 6], f32, kind="ExternalInput")
    w0_d = nc.dram_tensor("w0col", [3, 2], f32, kind="ExternalInput")
    out_d = nc.dram_tensor("out", [2, 3, L], f32, kind="ExternalOutput")

    with tile.TileContext(nc) as tc, ExitStack() as ctx:
        const = ctx.enter_context(tc.tile_pool(name="const", bufs=1))
        big = ctx.enter_context(tc.tile_pool(name="big", bufs=1))
        natp = ctx.enter_context(tc.tile_pool(name="natp", bufs=LT))
        ps = ctx.enter_context(tc.tile_pool(name="ps", bufs=1, space="PSUM"))
        # scoped pools (right side), freed mid-kernel:
        dpool_cm = tc.tile_pool(name="dpool", bufs=1, side="right")
        dpool = dpool_cm.__enter__()
        xtp_cm = tc.tile_pool(name="xtp", bufs=1, side="right")
        xtpp = xtp_cm.__enter__()
        nf32_cm = tc.tile_pool(name="nf32p", bufs=3, side="right")
        nf32p = nf32_cm.__enter__()
        stg_cm = tc.tile_pool(name="stg", bufs=1, side="right")
        stg = stg_cm.__enter__()

        def acc_t(name="acc"):
            return ps.tile([128, 1024], f32, tag="acc", name=name, bufs=2)

        def fmp_t(name="fmp"):
            return ps.tile([128, 512], f32, tag="fmp", name=name, bufs=4)

        # ---------------- constants ----------------
        identb = const.tile([128, 128], bf16, tag="identb")
        make_identity(nc, identb)
        onesb = const.tile([128, 128], bf16, tag="onesb")
        nc.vector.memset(onesb[:], 1.0)
        zerob = const.tile([128, 512], bf16, tag="zerob")
        nc.vector.memset(zerob[:], 0.0)
        w0sb = const.tile([3, 2], f32, tag="w0sb")
        nc.scalar.dma_start(w0sb[:], w0_d[:])

        wstg = stg.tile([128, 3 * 2 * 1024], f32, tag="stg_w", name="wstg",
                        bufs=1)
        nc.scalar.dma_start(
            wstg[:].rearrange("p (c j u) -> p c j u", c=3, j=2),
            wp_d[:].rearrange("c p j u -> p c j u"))
        Wp8 = xtpp.tile([128, 3 * 2 * 1024], fp8, tag="Wp8")
        nc.vector.tensor_copy(Wp8[:], wstg[:])
        Wp8v = Wp8[:].rearrange("p (c j u) -> p c j u", c=3, j=2)
        stg_cm.__exit__(None, None, None)

        bsb = const.tile([128, 6], f32, tag="bsb")
        nc.scalar.dma_start(bsb[:], bp_d[:])

        # ---------------- big SBUF tensors ----------------
        xT = [[big.tile([128, L], bf16, tag=f"xT{t}_{k}", name=f"xT{t}_{k}")
               for k in range(5)] for t in range(2)]
        xTp = [[xtpp.tile([128 if pc < 2 else 64, 2, L], fp8,
                          tag=f"xTp{t}_{pc}", name=f"xTp{t}_{pc}")
                for pc in range(3)] for t in range(2)]
        for t in range(2):
            nc.gpsimd.memset(xTp[t][2][:, 1, :], 0.0)
        dTP = [dpool.tile([128, 2, L], fp8, tag=f"dTP{t}", name=f"dTP{t}")
               for t in range(2)]
        dT2p = [dpool.tile([32, 2, L], fp8, tag=f"dT2p{t}", name=f"dT2p{t}")
                for t in range(2)]
        for t in range(2):
            nc.gpsimd.memset(dT2p[t][:, 1, :], 0.0)
        nats = [[None] * NP for _ in range(2)]      # main [128,2,512]
        natt = [[None] * NP for _ in range(2)]      # tail [128,2,128]

        # Pool-engine copy helper
        def pool_copy(out, in_):
            nc.gpsimd.tensor_copy(out, in_)

        # ---------------- phase 0: load, transpose, dense ----------------
        def p0_group(g, t):
            pjs2 = [ps.tile([128, 2, 512], bf16, tag="fmp", name="pjs",
                            bufs=4) for _ in range(3)]
            pjs = [pjs2[k // 2][:, k % 2, :] for k in range(5)]
            nf2s = []
            for pp in range(TG // 2):
                pi = g * (TG // 2) + pp
                nf2 = nf32p.tile([128, 2, D], f32, tag="nf", name="nf2")
                eng = nc.sync if pp % 2 == 0 else nc.scalar
                eng.dma_start(
                    nf2[:],
                    x_d[t, ds(pi * 256, 256), :].rearrange(
                        "(j k) d -> k j d", j=2))
                nf2s.append((pi, nf2))
                for j in range(2):
                    ii = pp * 2 + j
                    nfb = nf32p.tile([128, D], bf16, tag="nfb", name="nfb",
                                     bufs=4)
                    nc.vector.tensor_copy(nfb[:], nf2[:, j, :])
                    for k, (doff, dcnt) in enumerate(DCH):
                        nc.tensor.transpose(
                            pjs[k][:dcnt, ds(ii * 128, 128)],
                            nfb[:, ds(doff, dcnt)], identb[:])
            # evict transposes -> xT (bf16), then xT -> fp8 pairs; chunk
            # pipeline split across DVE/Pool so the dense chain advances
            # on both engines
            gsl = ds(g * GW, GW)
            xtp_dst = [(0, 0, 0, 128), (0, 1, 1, 128), (1, 0, 2, 128),
                       (1, 1, 3, 128), (2, 0, 4, 64)]
            for k, (doff, dcnt) in enumerate(DCH):
                if k in (1, 3):
                    nc.scalar.copy(xT[t][k][:dcnt, gsl], pjs[k][:dcnt, :])
                else:
                    nc.vector.tensor_copy(xT[t][k][:dcnt, gsl],
                                          pjs[k][:dcnt, :])
                pc, j, src, cnt = xtp_dst[k]
                cp = pool_copy if k in (2, 4) else nc.vector.tensor_copy
                cp(xTp[t][pc][:cnt, j, gsl], xT[t][src][:cnt, gsl])
                if k == 4:
                    nc.vector.tensor_copy(xTp[t][2][0:24, 1, gsl],
                                          xT[t][4][64:88, gsl])
            # dense DR matmuls for this column group; evictions are
            # relu+bias via STT on DVE/Pool (keeps ACT a pure exp stream)
            def relu_ev(dst, src, bias, on_act):
                if on_act:
                    nc.scalar.activation(dst, src, AF.Relu, bias=bias)
                else:
                    nc.vector.scalar_tensor_tensor(
                        dst, src, bias, zerob[:src.shape[0], :],
                        op0=ALU.add, op1=ALU.max)
            for m, (uoff, ucnt) in enumerate([(0, 128), (128, 128),
                                              (256, 44)]):
                accd = fmp_t(name="accd")
                for pc in range(3):
                    pp = 128 if pc < 2 else 64
                    nc.tensor.matmul(
                        accd[:ucnt, :],
                        Wp8v[:pp, pc, :, ds(t * U + uoff, ucnt)],
                        xTp[t][pc][:pp, :, gsl],
                        start=(pc == 0), stop=(pc == 2), perf_mode=DR)
                if m < 2:
                    relu_ev(dTP[t][:, m, gsl], accd[:ucnt, :],
                            bsb[:ucnt, t * 3 + m: t * 3 + m + 1],
                            on_act=(m == 1))
                else:
                    relu_ev(dT2p[t][0:32, 0, gsl], accd[0:32, :],
                            bsb[0:32, t * 3 + m: t * 3 + m + 1],
                            on_act=False)
                    relu_ev(dT2p[t][0:12, 1, gsl], accd[32:44, :],
                            bsb[32:44, t * 3 + m: t * 3 + m + 1],
                            on_act=True)
            # nats builds last: off the dense-affinity critical chain
            for pi, nf2 in nf2s:
                nt = natp.tile([128, 2, 512], fp8, tag="nat",
                               name=f"nat{t}_{pi}")
                tl = natp.tile([128, 2, 128], fp8, tag="ntl",
                               name=f"ntl{t}_{pi}")
                nats[t][pi] = nt
                natt[t][pi] = tl
                pool_copy(nt[:], nf2[:, :, 0:512])
                nc.vector.tensor_copy(tl[:, :, 0:88], nf2[:, :, 512:D])
                nc.gpsimd.memset(tl[:, :, 88:ONES_COL], 0.0)
                nc.gpsimd.memset(tl[:, :, ONES_COL + 1:128], 0.0)
                nc.gpsimd.memset(tl[:, :, ONES_COL:ONES_COL + 1], 1.0)

        # ---------------- phase 1: affinity -> E (both layouts) ----------
        def e_tiles(tag):
            return [epool.tile([128, 2, L], fp8, tag="E", name=f"E{tag}_{pi}")
                    for pi in range(NP)]

        def emit_e_unit(a, b, E, i, h):
            """One (l-tile, half) of E = exp(SCALE * dense_a.T @ dense_b)."""
            e = E[i // 2]
            ej = i % 2
            isl = ds(i * 128, 128)
            acc = acc_t(name="eacc")
            for sx in range(2):
                nsl = ds(h * 1024 + sx * 512, 512)
                asl = ds(sx * 512, 512)
                nc.tensor.matmul(acc[:, asl], dTP[a][:, :, isl],
                                 dTP[b][:, :, nsl],
                                 start=True, stop=False, perf_mode=DR)
                nc.tensor.matmul(acc[:, asl], dT2p[a][:, :, isl],
                                 dT2p[b][:, :, nsl],
                                 start=False, stop=True, perf_mode=DR)
            nc.scalar.activation(e[:, ej, ds(h * 1024, 1024)],
                                 acc[:, :], AF.Exp, scale=SCALE)

        # ---------------- aligned + FM per side ----------------
        def aligned_T(s, E, side_tag, hook=None, r_on_act=True,
                      psa_on_acc=False):
            """alT[k] [d, L] bf16 = normalized aligned.T."""
            nat = nats[s]
            ntl = natt[s]
            # pass A: d 512:600 + ones row
            if psa_on_acc:
                psAt = [acc_t(name="psA") for _ in range(2)]
                psA = [psAt[nx // 2][:, ds((nx % 2) * 512, 512)]
                       for nx in range(4)]
            else:
                psA = [fmp_t(name="psA") for _ in range(4)]
            for pi in range(NP):
                for nx in range(4):
                    nc.tensor.matmul(psA[nx][:, :],
                                     ntl[pi][:],
                                     E[pi][:, :, ds(nx * 512, 512)],
                                     start=(pi == 0), stop=(pi == NP - 1),
                                     perf_mode=DR)
            # R chain
            R = big.tile([128, L], bf16, tag="R", name=f"R{side_tag}")
            for h in range(NH):
                rr = rp.tile([128, 1024], f32, tag="rr", name="rr")
                rrb = rp.tile([128, 1024], bf16, tag="rrb", name="rrb")
                for sx in range(2):
                    nc.vector.reciprocal(
                        rr[96:97, ds(sx * 512, 512)],
                        psA[h * 2 + sx][ONES_ROW:ONES_ROW + 1, :])
                nc.vector.tensor_copy(rrb[96:97, :], rr[96:97, :])
                # bc must come from the OTHER psum ring than psA (psA slots
                # are all live until the alT4 eviction, which needs R)
                if psa_on_acc:
                    for sx in range(2):
                        bcx = fmp_t(name="bc")
                        nc.tensor.matmul(bcx[:, :], onesb[96:97, 0:128],
                                         rrb[96:97, ds(sx * 512, 512)],
                                         start=True, stop=True,
                                         tile_position=(96, 0))
                        if r_on_act:
                            nc.scalar.copy(
                                R[:, ds(h * 1024 + sx * 512, 512)], bcx[:, :])
                        else:
                            nc.vector.tensor_copy(
                                R[:, ds(h * 1024 + sx * 512, 512)], bcx[:, :])
                else:
                    bc = acc_t(name="bc")
                    for sx in range(2):
                        nc.tensor.matmul(bc[:, ds(sx * 512, 512)],
                                         onesb[96:97, 0:128],
                                         rrb[96:97, ds(sx * 512, 512)],
                                         start=True, stop=True,
                                         tile_position=(96, 0))
                    if r_on_act:
                        nc.scalar.copy(R[:, ds(h * 1024, 1024)], bc[:, :])
                    else:
                        nc.vector.tensor_copy(R[:, ds(h * 1024, 1024)],
                                              bc[:, :])
            alT = [alp.tile([128, L], bf16, tag=f"alT{k}",
                            name=f"alT{side_tag}{k}") for k in range(5)]
            # evict pass A (d-chunk 4)
            for nx in range(4):
                nsl = ds(nx * 512, 512)
                nc.vector.tensor_mul(alT[4][0:88, nsl], psA[nx][0:88, :],
                                     R[0:88, nsl])
            # passes m=0..3 (hook interleaves independent PE work)
            for m in range(4):
                for h in range(NH):
                    acc = acc_t(name="alacc")
                    for pi in range(NP):
                        for sx in range(2):
                            asl = ds(sx * 512, 512)
                            nsl = ds(h * 1024 + sx * 512, 512)
                            nc.tensor.matmul(acc[:, asl],
                                             nat[pi][:, :, ds(m * 128, 128)],
                                             E[pi][:, :, nsl],
                                             start=(pi == 0),
                                             stop=(pi == NP - 1),
                                             perf_mode=DR)
                    hsl = ds(h * 1024, 1024)
                    nc.vector.tensor_mul(alT[m][:, hsl], acc[:, :], R[:, hsl])
                if hook is not None:
                    hook(m)
            return alT

        def prebuild_b2(s, xTs, on_pool, nk=4):
            """Pair-b tiles (b^2, xb^2) with the b^2 row built early."""
            PBt = []
            for k, (doff, dcnt) in enumerate(DCH[:nk]):
                PB = fmbb.tile([128, 2, L], fp8, tag="PBb", name=f"PBb{s}_{k}")
                b_ = xTs[k][:dcnt, :]
                if on_pool == 'pool':
                    nc.gpsimd.tensor_mul(PB[:dcnt, 0, :], b_, b_)
                elif on_pool == 'dve':
                    nc.vector.tensor_mul(PB[:dcnt, 0, :], b_, b_)
                else:
                    nc.scalar.activation(PB[:dcnt, 0, :], b_, AF.Square)
                PBt.append(PB)
            return PBt

        def fm_side(s, alT, xTs, PBt):
            """FM heads for side s: x = alT (aligned), b = xTs (raw).

            Pair-a = (x^2, xb), pair-b = (b^2, xb^2).  Both DR matmuls
            accumulate into the same base-0 psum rows (disjoint stationary
            columns); X/B bf16 groups are nx-packed at positions 0/32/64/96.
            """
            sk = lambda k: s * 5 + k
            PAt = []
            for k, (doff, dcnt) in enumerate(DCH):
                if k >= len(PBt):
                    PBb = fmbb.tile([128, 2, L], fp8, tag="PBb",
                                    name=f"PBb{s}_{k}")
                    b2_ = xTs[k][:dcnt, :]
                    if s == 0:
                        nc.gpsimd.tensor_mul(PBb[:dcnt, 0, :], b2_, b2_)
                    else:
                        nc.scalar.activation(PBb[:dcnt, 0, :], b2_, AF.Square)
                    PBt.append(PBb)
                PA = fma.tile([128, 2, L], fp8, tag="PAa", name=f"PAa{s}_{k}")
                PBb = PBt[k]
                x_ = alT[k][:dcnt, :]
                b_ = xTs[k][:dcnt, :]
                if s == 0:
                    nc.gpsimd.tensor_mul(PA[:dcnt, 0, :], x_, x_)
                    nc.vector.tensor_mul(PA[:dcnt, 1, :], x_, b_)
                    nc.scalar.activation(PBb[:dcnt, 1, :], PA[:dcnt, 1, :],
                                         AF.Square)
                else:
                    nc.scalar.activation(PA[:dcnt, 0, :], x_, AF.Square)
                    nc.vector.tensor_mul(PA[:dcnt, 1, :], x_, b_)
                    nc.scalar.activation(PBb[:dcnt, 1, :], PA[:dcnt, 1, :],
                                         AF.Square)
                PAt.append(PA)
            # projections: AB nx-packed in 2 fmp tiles; pair-a/pair-b DR
            # into base-0 rows of 2 acc tiles (one 512-half per nx)
            ABt = [fmp_t(name=f"ABt{i}") for i in range(2)]
            BC = [acc_t(name=f"BC{i}") for i in range(2)]
            for k, (doff, dcnt) in enumerate(DCH):
                fl = (k == 0, k == 4)
                for nx in range(4):
                    nsl = ds(nx * 512, 512)
                    AB = ABt[nx // 2]
                    pb = (nx % 2) * 64
                    nc.tensor.matmul(AB[pb:pb + 32, :],
                                     pa_stat[s][k][:dcnt, 0:32],
                                     alT[k][:dcnt, nsl],
                                     start=fl[0], stop=fl[1],
                                     tile_position=(0, pb),
                                     skip_group_check=True)
                    nc.tensor.matmul(AB[pb + 32:pb + 64, :],
                                     pa_stat[s][k][:dcnt, 32:64],
                                     xTs[k][:dcnt, nsl],
                                     start=fl[0], stop=fl[1],
                                     tile_position=(0, pb + 32),
                                     skip_group_check=True)
                    hsl = ds((nx % 2) * 512, 512)
                    nc.tensor.matmul(BC[nx // 2][0:32, hsl],
                                     PBCv[:dcnt, sk(k), :, 0:32],
                                     PAt[k][:dcnt, :, nsl],
                                     start=fl[0], stop=False,
                                     perf_mode=DR, skip_group_check=True)
                    nc.tensor.matmul(BC[nx // 2][0:32, hsl],
                                     PBCv[:dcnt, sk(k), :, 32:64],
                                     PBt[k][:dcnt, :, nsl],
                                     start=False, stop=fl[1],
                                     perf_mode=DR, skip_group_check=True)
            # combine
            S = sp.tile([128, L], bf16, tag="S", name=f"S{s}")
            TAs = sp.tile([16, L], bf16, tag="TAs", name=f"TAs{s}")
            for nx in range(4):
                nsl = ds(nx * 512, 512)
                AB = ABt[nx // 2]
                pb = (nx % 2) * 64
                hsl = ds((nx % 2) * 512, 512)
                nc.vector.tensor_copy(S[0:64, nsl], AB[pb:pb + 64, :])
                nc.vector.tensor_copy(S[64:96, nsl], BC[nx // 2][0:32, hsl])
                nc.vector.tensor_add(TAs[0:10, nsl], AB[pb:pb + 10, :],
                                     S[32:42, nsl])
                # M^2 -> S[96:101] (outside the copied range): psum x the
                # raw bf16 copy in S[64:69] (only one PSUM input allowed)
                if s == 0:
                    nc.vector.tensor_mul(S[96:101, nsl],
                                         BC[nx // 2][0:5, hsl],
                                         S[64:69, nsl])
                else:
                    nc.scalar.activation(S[96:101, nsl],
                                         BC[nx // 2][0:5, hsl], AF.Square)
            if s == 0:
                nc.vector.tensor_mul(S[0:10, :], TAs[0:10, :], TAs[0:10, :])
            else:
                nc.scalar.activation(S[0:10, :], TAs[0:10, :], AF.Square)
            for nx in range(4):
                nsl = ds(nx * 512, 512)
                cps = fmp_t(name="cps")
                nc.tensor.matmul(cps[0:3, :], cb2[0:101, ds(s * 3, 3)],
                                 S[0:101, nsl], start=True, stop=True)
                o = ob.tile([3, 512], f32, tag="ob", name="o")
                if s == 0:
                    nc.vector.scalar_tensor_tensor(
                        o[:, :], cps[0:3, :], w0sb[:, s:s + 1],
                        zerob[0:3, :], op0=ALU.add, op1=ALU.add)
                else:
                    nc.scalar.activation(o[:, :], cps[0:3, :], AF.Identity,
                                         bias=w0sb[:, s:s + 1])
                nc.sync.dma_start(out_d[s, :, nsl], o[:, :])

        # ---------------- main flow ----------------
        lstg_cm = tc.tile_pool(name="lstg", bufs=1, side="right")
        lstg = lstg_cm.__enter__()
        pstg = lstg.tile([128, 10 * 64], f32, tag="stg_pa", name="pstg",
                         bufs=1)
        nc.scalar.dma_start(
            pstg[:].rearrange("p (t c) -> p t c", t=10),
            pa_d[:].rearrange("t p c -> p t c"))
        PAst = const.tile([128, 10 * 64], bf16, tag="PAst")
        nc.vector.tensor_copy(PAst[:], pstg[:])
        pa_stat = [[PAst[:, ds((s * 5 + k) * 64, 64)] for k in range(5)]
                   for s in range(2)]
        bstg = lstg.tile([128, 10 * 2 * 64], f32, tag="stg_pbc", name="bstg",
                         bufs=1)
        nc.scalar.dma_start(
            bstg[:].rearrange("p (t j c) -> p t j c", t=10, j=2),
            pbc_d[:].rearrange("t p j c -> p t j c"))
        PBCst = const.tile([128, 10 * 2 * 64], fp8, tag="PBCst")
        nc.vector.tensor_copy(PBCst[:], bstg[:])
        PBCv = PBCst[:].rearrange("p (t j c) -> p t j c", t=10, j=2)
        cstg = lstg.tile([128, 6], f32, tag="stg_c", name="cstg", bufs=1)
        nc.scalar.dma_start(cstg[:], c2_d[:])
        cb2 = const.tile([128, 6], bf16, tag="cb2")
        nc.vector.tensor_copy(cb2[:], cstg[:])
        stg_cm.__exit__(None, None, None)
        epool = ctx.enter_context(tc.tile_pool(name="epool", bufs=LT))
        E1 = e_tiles("1")
        E2 = e_tiles("2")

        def e_batch(g):
            # all affinity units whose dense column groups are complete
            units = []
            if g >= 1:
                hs = [0] if g < 3 else [0, 1]
                for h in hs:
                    if h == 1:
                        iset = range(LT)
                    else:
                        iset = (range((g + 1) * TG) if g < 3
                                else range(2 * TG, LT))
                    for i in iset:
                        units.append((i, h))
            # E1 only (aligned1 is the nearest consumer); E2 units are
            # deferred so the ACT queue drains all E1 exps first.
            for i, h in units:
                if (i, h) not in emitted1:
                    emitted1.add((i, h))
                    emit_e_unit(0, 1, E1, i, h)
                    deferred2.append((i, h))

        emitted1 = set()
        deferred2 = []
        for g in range(NG):
            p0_group(g, 0)
            p0_group(g, 1)
            e_batch(g)

        nf32_cm.__exit__(None, None, None)
        xtp_cm.__exit__(None, None, None)
        alp = ctx.enter_context(tc.tile_pool(name="alp", bufs=1))
        rp = ctx.enter_context(tc.tile_pool(name="rp", bufs=1))

        # E2 units in tile-major order, interleaved into aligned1's m-loop
        # (the PSUM acc ring is allocation-ordered, so emitting them after
        # each m-pass lets aligned1 run as soon as E1 is ready while E2
        # trails on ACT).
        e2units = [(2 * pi + j, h)
                   for pi in range(NP) for j in range(2) for h in range(NH)]

        def e2_hook(m):
            for i, h in e2units[m * 8:(m + 1) * 8]:
                emit_e_unit(1, 0, E2, i, h)

        fma = ctx.enter_context(tc.tile_pool(name="fma", bufs=3))
        fmbb = ctx.enter_context(tc.tile_pool(name="fmbb", bufs=5))
        # side-0 b^2 prebuilt on Pool (fills the E1-exp window)
        PB0 = prebuild_b2(0, xT[0], on_pool='dve')
        qaT = aligned_T(1, E1, "q", hook=e2_hook, r_on_act=False)
        dpool_cm.__exit__(None, None, None)
        sp = ctx.enter_context(tc.tile_pool(name="sp", bufs=1))
        ob = ctx.enter_context(tc.tile_pool(name="ob", bufs=4))
        fm_side(0, qaT, xT[0], PB0)
        PB1 = prebuild_b2(1, xT[1], on_pool='act')
        paT = aligned_T(0, E2, "p", psa_on_acc=True)
        fm_side(1, paT, xT[1], PB1)


def _host_prep(W1, b1, W2, b2, cat_w0, cat_w, cat_V, dm_w0, dm_w, dm_V):
    # dense pair weights: wpair[pc][k][j] = W_t rows; pc<2: d=pc*256+j*128+k
    # pc=2 (44 partitions): d = 512 + j*44 + k
    wpair = np.zeros((3, 128, 2, 1024), np.float32)
    for t, W in enumerate((W1, W2)):
        for pc in range(2):
            for j in range(2):
                d0 = pc * 256 + j * 128
                wpair[pc, :, j, t * U:(t + 1) * U] = W[d0:d0 + 128]
        wpair[2, 0:64, 0, t * U:(t + 1) * U] = W[512:576]
        wpair[2, 0:24, 1, t * U:(t + 1) * U] = W[576:600]

    # PA stationaries (bf16): cols 0:12 x-side, 32:44 b-side (padded to 32/64)
    pastat = np.zeros((10, 128, 64), np.float32)
    # PB/PC stationaries (fp8 pairs)
    pbcstat = np.zeros((10, 128, 2, 64), np.float32)
    for s in range(2):
        ci, di, mi = s, s, s + 2
        Va = cat_V[ci][:, :D]
        Vb = cat_V[ci][:, D:]
        Vd = dm_V[di]
        Vm = dm_V[mi]
        ua = (Va ** 2).sum(0) * USC
        ub = (Vb ** 2).sum(0) * USC
        ud = (Vd ** 2).sum(0) * USC
        um = (Vm ** 2).sum(0) * USC
        xs = np.zeros((D, 64), np.float32)
        xs[:, 0:5] = Va.T
        xs[:, 5:10] = Vd.T
        xs[:, 10] = cat_w[ci, :D]
        xs[:, 11] = dm_w[di]
        xs[:, 32 + 0:32 + 5] = Vb.T
        xs[:, 32 + 5:32 + 10] = -Vd.T
        xs[:, 32 + 10] = cat_w[ci, D:]
        xs[:, 32 + 11] = dm_w[di]
        bs = np.zeros((D, 2, 64), np.float32)
        # pair-a = (x^2, xb): j0 -> x^2 stats, j1 -> xb stats
        bs[:, 0, 5] = ua
        bs[:, 0, 6] = ud
        bs[:, 1, 0:5] = Vm.T
        bs[:, 1, 7] = dm_w[mi]
        bs[:, 1, 8] = ud
        # pair-b = (b^2, xb^2): j0 -> b^2 stats, j1 -> xb^2 stats
        bs[:, 0, 32 + 9] = ub
        bs[:, 0, 32 + 10] = ud
        bs[:, 1, 32 + 11] = um
        for k, (doff, dcnt) in enumerate(DCH):
            pastat[s * 5 + k, :dcnt] = xs[doff:doff + dcnt]
            pbcstat[s * 5 + k, :dcnt] = bs[doff:doff + dcnt]

    # combine matrix: S rows -> 3 outputs per side
    comb2 = np.zeros((128, 6), np.float32)
    for s in range(2):
        C = comb2[:, s * 3:(s + 1) * 3]
        C[10, 0] = 1.0          # x@w_cat
        C[42, 0] = 1.0          # b@w_cat2
        C[0:5, 0] = 0.5         # cat quads (TA^2 rows)
        C[69, 0] = -0.5 / USC   # x2@ua
        C[73, 0] = -0.5 / USC   # b2@ub
        C[11, 1] = 1.0          # x@w_d
        C[43, 1] = -1.0         # -b@w_d
        C[5:10, 1] = 0.5        # diff quads (TA^2 rows)
        C[70, 1] = -0.5 / USC   # x2@ud
        C[74, 1] = -0.5 / USC   # b2@ud
        C[72, 1] = 1.0 / USC    # xb@ud
        C[71, 2] = 1.0          # xb@w_m
        C[96:101, 2] = 0.5      # mul quads (M^2 rows)
        C[75, 2] = -0.5 / USC   # xb2@um

    biasp = np.zeros((128, 6), np.float32)
    for t, b in enumerate((b1, b2)):
        for m, (uoff, ucnt) in enumerate([(0, 128), (128, 128), (256, 44)]):
            if m < 2:
                biasp[:ucnt, t * 3 + m] = b[uoff:uoff + ucnt]
            else:
                biasp[0:32, t * 3 + m] = b[256:288]
                biasp[32:44, t * 3 + m] = b[288:300]

    w0col = np.zeros((3, 2), np.float32)
    for s in range(2):
        w0col[0, s] = cat_w0[s, 0]
        w0col[1, s] = dm_w0[s, 0]
        w0col[2, s] = dm_w0[s + 2, 0]
    return wpair, pastat, pbcstat, comb2, biasp, w0col


_PROG = None


def _get_prog():
    global _PROG
    if _PROG is None:
        from concourse import bacc
        nc = bacc.Bacc(None, target_bir_lowering=False)
        _emit(nc, L_FULL)
        nc.finalize()
        _PROG = nc
    return _PROG


def _in_maps(stack_input, W1, b1, W2, b2, fm_cat_w0, fm_cat_w, fm_cat_V,
             fm_dm_w0, fm_dm_w, fm_dm_V):
    f = lambda a: np.ascontiguousarray(np.asarray(a, np.float32))
    stack_input = f(stack_input)
    wpair, pastat, pbcstat, comb2, biasp, w0col = _host_prep(
        f(W1), f(b1), f(W2), f(b2), f(fm_cat_w0), f(fm_cat_w), f(fm_cat_V),
        f(fm_dm_w0), f(fm_dm_w), f(fm_dm_V))
    common = {"wpair": wpair, "pastat": pastat, "pbcstat": pbcstat,
              "comb2": comb2, "biasp": biasp, "w0col": w0col}
    return [dict(common, x=np.ascontiguousarray(stack_input[:, b]))
            for b in range(N_CORES)]


def kernel(stack_input, W1, b1, W2, b2, fm_cat_w0, fm_cat_w, fm_cat_V,
           fm_dm_w0, fm_dm_w, fm_dm_V):
    from concourse.bass_utils import run_bass_kernel_spmd

    in_maps = _in_maps(stack_input, W1, b1, W2, b2, fm_cat_w0, fm_cat_w,
                       fm_cat_V, fm_dm_w0, fm_dm_w, fm_dm_V)
    nc = _get_prog()
    res = run_bass_kernel_spmd(nc, in_maps, core_ids=list(range(N_CORES)))
    outs = [r["out"] for r in res.results]            # each [2, 3, L]
    fp = np.stack([o[0].T for o in outs]).astype(np.float32)   # [8, L, 3]
    fq = np.stack([o[1].T for o in outs]).astype(np.float32)
    return fp, fq
